# revision 8
# baseline (speedup 1.0000x reference)
"""BitNet-style row-parallel linear on 8 TRN2 NeuronCores.

Reference computes: out[b,s,o] = sum_d x[b,s,d] * sign(w[o,d]) + bias[o]
  x: [4, 2048, 4096] f32, w: [4096, 4096] f32, bias: [4096] f32.

Strategy: data-parallel over the 8192 (b*s) rows — each of the 8 cores
computes a 1024-row slice of the output against the full binarized
weight. No collective needed; shards concatenate to the full output.
(The row-parallel/all-reduce hint costs a 128MB all-reduce per core;
sharding M instead makes the partial outputs disjoint.)

TensorE consumes both operands K-major, so the host preps:
  kxm = x_shard.T           [K=4096, M=1024]  (per core)
  kxn = sign(w).T           [K=4096, N=4096]  (same on every core)
Matmul runs in float32r (fp22 multiply, fp32 accumulate) — 4x faster
than true fp32 on the PE and far more accurate than bf16 inputs.
"""

import numpy as np

B, S, D_IN, D_OUT = 4, 2048, 4096, 4096
NCORES = 8
M_TOTAL = B * S
M_CORE = M_TOTAL // NCORES

import os

_cache = {}

# "f32r" (fp22 multiply, highest precision) or "bf16" (half the DMA
# traffic + fast weight load; weights are exactly representable).
DTYPE = os.environ.get("BK_DTYPE", "bf16")


IMPL = os.environ.get("BK_IMPL", "custom")


def _custom_body(nc, tc, kxm, kxn, out, mm_dt, mybir):
    """x^T stays SBUF-resident; sign(w)^T streams through once.

    Per n-block of 512 output columns: accumulate all 32 k-tiles into
    8 PSUM banks (one per 128-row m-tile), in two halves of 4 banks so
    each half's eviction overlaps the other half's matmuls.
    """
    P = 128
    KT = D_IN // P          # 32 k tiles
    MT = M_CORE // P        # 8 m tiles
    NW = 512
    NB = D_OUT // NW        # 8 n blocks
    CH = 8                  # k tiles per kxn DMA chunk
    f32 = mybir.dt.float32

    from contextlib import ExitStack
    with ExitStack() as ctx:
        kxm_pool = ctx.enter_context(tc.tile_pool(name="kxm", bufs=1))
        kxn_pool = ctx.enter_context(tc.tile_pool(name="kxn", bufs=8))
        psum_pool = ctx.enter_context(
            tc.tile_pool(name="psum", bufs=8, space="PSUM"))
        out_pool = ctx.enter_context(tc.tile_pool(name="outp", bufs=8))
        kxm_tiles = []
        for k in range(KT):
            kt = kxm_pool.tile([P, M_CORE], mm_dt, tag="kxm",
                               name=f"kxm_{k}", bufs=KT)
            nc.sync.dma_start(out=kt[:, :], in_=kxm[k * P:(k + 1) * P, :])
            kxm_tiles.append(kt)

        for nb in range(NB):
            ncols = slice(nb * NW, (nb + 1) * NW)
            chunks = []
            for c in range(KT // CH):
                t = kxn_pool.tile([P, CH, NW], mm_dt, tag="kxn", name=f"kxn_{nb}_{c}")
                src = kxn[c * CH * P:(c + 1) * CH * P, ncols]
                nc.sync.dma_start(
                    out=t, in_=src.rearrange("(ko ki) n -> ki ko n", ki=P))
                chunks.append(t)
            psums = [psum_pool.tile([P, NW], f32, tag="ps", name=f"ps_{nb}_{i}")
                     for i in range(MT)]
            for half in range(2):
                ms = range(half * (MT // 2), (half + 1) * (MT // 2))
                for k in range(KT):
                    rhs = chunks[k // CH][:, k % CH, :]
                    for m in ms:
                        nc.tensor.matmul(
                            psums[m][:, :],
                            lhsT=kxm_tiles[k][:, m * P:(m + 1) * P],
                            rhs=rhs,
                            start=(k == 0), stop=(k == KT - 1))
                for m in ms:
                    ot = out_pool.tile([P, NW], f32, tag="ot", name=f"ot_{nb}_{m}")
                    nc.vector.tensor_copy(out=ot[:, :], in_=psums[m][:, :])
                    nc.gpsimd.dma_start(
                        out=out[m * P:(m + 1) * P, ncols], in_=ot[:, :])


def _build():
    """Build + compile the 8-core SPMD Bass program once per process."""
    if "nc" in _cache:
        return _cache["nc"]

    import concourse.bacc as bacc
    import concourse.tile as tile
    import concourse.mybir as mybir
    from concourse.kernels.tile_matmul import matmul_tile_kernel

    mm_dt = {"f32r": mybir.dt.float32r, "bf16": mybir.dt.bfloat16}[DTYPE]

    nc = bacc.Bacc("TRN2", target_bir_lowering=False, debug=False,
                   num_devices=NCORES)
    kxm = nc.dram_tensor("kxm", [D_IN, M_CORE], mm_dt,
                         kind="ExternalInput").ap()
    kxn = nc.dram_tensor("kxn", [D_IN, D_OUT], mm_dt,
                         kind="ExternalInput").ap()
    out = nc.dram_tensor("out", [M_CORE, D_OUT], mybir.dt.float32,
                         kind="ExternalOutput").ap()
    if IMPL == "custom":
        with tile.TileContext(nc) as tc:
            _custom_body(nc, tc, kxm, kxn, out, mm_dt, mybir)
    else:
        kw = {}
        if os.environ.get("BK_MAX_K_TILE"):
            kw["MAX_K_TILE_SIZE"] = int(os.environ["BK_MAX_K_TILE"])
        if os.environ.get("BK_SKIP_K_SNAKE"):
            kw["skip_k_snake"] = True
        if os.environ.get("BK_NO_CACHE_TILES"):
            kw["cache_tiles"] = False
        with tile.TileContext(nc) as tc:
            matmul_tile_kernel(tc, kxm, kxn, out, **kw)
    nc.compile()
    _cache["nc"] = nc
    return nc


def _prep_inputs(x, weight):
    if DTYPE == "bf16":
        import ml_dtypes
        np_dt = ml_dtypes.bfloat16
    else:
        np_dt = np.float32
    x2d = np.asarray(x, dtype=np.float32).reshape(M_TOTAL, D_IN)
    kxn = np.ascontiguousarray(np.sign(weight, dtype=np.float32).T.astype(np_dt))
    in_maps = []
    for c in range(NCORES):
        kxm = np.ascontiguousarray(x2d[c * M_CORE:(c + 1) * M_CORE].T.astype(np_dt))
        in_maps.append({"kxm": kxm, "kxn": kxn})
    return in_maps


def _run(x, weight, bias, trace=False):
    from concourse.bass_utils import run_bass_kernel_spmd

    nc = _build()
    in_maps = _prep_inputs(x, weight)
    res = run_bass_kernel_spmd(nc, in_maps, core_ids=list(range(NCORES)),
                               trace=trace)
    out = np.concatenate([res.results[c]["out"] for c in range(NCORES)],
                         axis=0)
    bias = np.asarray(bias, dtype=np.float32)
    if np.any(bias):
        out += bias
    return out.reshape(B, S, D_OUT), res


def kernel(x, weight, bias):
    out, _ = _run(x, weight, bias, trace=False)
    return out


# revision 9
# speedup vs baseline: 1.0209x; 1.0209x over previous
"""BitNet-style row-parallel linear on 8 TRN2 NeuronCores.

Reference computes: out[b,s,o] = sum_d x[b,s,d] * sign(w[o,d]) + bias[o]
  x: [4, 2048, 4096] f32, w: [4096, 4096] f32, bias: [4096] f32.

Strategy: data-parallel over the 8192 (b*s) rows — each of the 8 cores
computes a 1024-row slice of the output against the full binarized
weight. No collective needed; shards concatenate to the full output.
(The row-parallel/all-reduce hint costs a 128MB all-reduce per core;
sharding M instead makes the partial outputs disjoint.)

TensorE consumes both operands K-major, so the host preps:
  kxm = x_shard.T           [K=4096, M=1024]  (per core)
  kxn = sign(w).T           [K=4096, N=4096]  (same on every core)
Matmul runs in float32r (fp22 multiply, fp32 accumulate) — 4x faster
than true fp32 on the PE and far more accurate than bf16 inputs.
"""

import numpy as np

B, S, D_IN, D_OUT = 4, 2048, 4096, 4096
NCORES = 8
M_TOTAL = B * S
M_CORE = M_TOTAL // NCORES

import os

_cache = {}

# "f32r" (fp22 multiply, highest precision) or "bf16" (half the DMA
# traffic + fast weight load; weights are exactly representable).
DTYPE = os.environ.get("BK_DTYPE", "bf16")


IMPL = os.environ.get("BK_IMPL", "custom")


def _custom_body(nc, tc, kxm, kxn, out, mm_dt, mybir):
    """x^T stays SBUF-resident; sign(w)^T streams through once.

    Per n-block of 512 output columns: accumulate all 32 k-tiles into
    8 PSUM banks (one per 128-row m-tile), in two halves of 4 banks so
    each half's eviction overlaps the other half's matmuls.
    """
    P = 128
    KT = D_IN // P          # 32 k tiles
    MT = M_CORE // P        # 8 m tiles
    NW = 512
    NB = D_OUT // NW        # 8 n blocks
    CH = 4                  # k tiles per kxn DMA chunk
    NC = KT // CH           # chunks per n block
    f32 = mybir.dt.float32

    from contextlib import ExitStack
    with ExitStack() as ctx:
        kxm_pool = ctx.enter_context(tc.tile_pool(name="kxm", bufs=1))
        kxn_pool = ctx.enter_context(tc.tile_pool(name="kxn", bufs=16))
        psum_pool = ctx.enter_context(
            tc.tile_pool(name="psum", bufs=8, space="PSUM"))
        out_pool = ctx.enter_context(tc.tile_pool(name="outp", bufs=8))

        def issue_chunks(nb):
            ncols = slice(nb * NW, (nb + 1) * NW)
            chunks = []
            for c in range(NC):
                t = kxn_pool.tile([P, CH, NW], mm_dt, tag="kxn",
                                  name=f"kxn_{nb}_{c}")
                src = kxn[c * CH * P:(c + 1) * CH * P, ncols]
                nc.sync.dma_start(
                    out=t, in_=src.rearrange("(ko ki) n -> ki ko n", ki=P))
                chunks.append(t)
            return chunks

        # Weight chunks for block 0 go first so the first matmul isn't
        # queued behind the full 8MB x load; x loads use the scalar
        # queue to stay off the weight stream's queue entirely.
        next_chunks = issue_chunks(0)
        kxm_tiles = []
        for k in range(KT):
            kt = kxm_pool.tile([P, M_CORE], mm_dt, tag="kxm",
                               name=f"kxm_{k}", bufs=KT)
            nc.scalar.dma_start(out=kt[:, :], in_=kxm[k * P:(k + 1) * P, :])
            kxm_tiles.append(kt)

        for nb in range(NB):
            ncols = slice(nb * NW, (nb + 1) * NW)
            chunks = next_chunks
            psums = [psum_pool.tile([P, NW], f32, tag="ps", name=f"ps_{nb}_{i}")
                     for i in range(MT)]
            for half in range(2):
                ms = range(half * (MT // 2), (half + 1) * (MT // 2))
                for k in range(KT):
                    rhs = chunks[k // CH][:, k % CH, :]
                    for m in ms:
                        nc.tensor.matmul(
                            psums[m][:, :],
                            lhsT=kxm_tiles[k][:, m * P:(m + 1) * P],
                            rhs=rhs,
                            start=(k == 0), stop=(k == KT - 1))
                if half == 0 and nb + 1 < NB:
                    next_chunks = issue_chunks(nb + 1)
                for m in ms:
                    ot = out_pool.tile([P, NW], f32, tag="ot", name=f"ot_{nb}_{m}")
                    nc.vector.tensor_copy(out=ot[:, :], in_=psums[m][:, :])
                    nc.gpsimd.dma_start(
                        out=out[m * P:(m + 1) * P, ncols], in_=ot[:, :])


def _build():
    """Build + compile the 8-core SPMD Bass program once per process."""
    if "nc" in _cache:
        return _cache["nc"]

    import concourse.bacc as bacc
    import concourse.tile as tile
    import concourse.mybir as mybir
    from concourse.kernels.tile_matmul import matmul_tile_kernel

    mm_dt = {"f32r": mybir.dt.float32r, "bf16": mybir.dt.bfloat16}[DTYPE]

    nc = bacc.Bacc("TRN2", target_bir_lowering=False, debug=False,
                   num_devices=NCORES)
    kxm = nc.dram_tensor("kxm", [D_IN, M_CORE], mm_dt,
                         kind="ExternalInput").ap()
    kxn = nc.dram_tensor("kxn", [D_IN, D_OUT], mm_dt,
                         kind="ExternalInput").ap()
    out = nc.dram_tensor("out", [M_CORE, D_OUT], mybir.dt.float32,
                         kind="ExternalOutput").ap()
    if IMPL == "custom":
        with tile.TileContext(nc) as tc:
            _custom_body(nc, tc, kxm, kxn, out, mm_dt, mybir)
    else:
        kw = {}
        if os.environ.get("BK_MAX_K_TILE"):
            kw["MAX_K_TILE_SIZE"] = int(os.environ["BK_MAX_K_TILE"])
        if os.environ.get("BK_SKIP_K_SNAKE"):
            kw["skip_k_snake"] = True
        if os.environ.get("BK_NO_CACHE_TILES"):
            kw["cache_tiles"] = False
        with tile.TileContext(nc) as tc:
            matmul_tile_kernel(tc, kxm, kxn, out, **kw)
    nc.compile()
    _cache["nc"] = nc
    return nc


def _prep_inputs(x, weight):
    if DTYPE == "bf16":
        import ml_dtypes
        np_dt = ml_dtypes.bfloat16
    else:
        np_dt = np.float32
    x2d = np.asarray(x, dtype=np.float32).reshape(M_TOTAL, D_IN)
    kxn = np.ascontiguousarray(np.sign(weight, dtype=np.float32).T.astype(np_dt))
    in_maps = []
    for c in range(NCORES):
        kxm = np.ascontiguousarray(x2d[c * M_CORE:(c + 1) * M_CORE].T.astype(np_dt))
        in_maps.append({"kxm": kxm, "kxn": kxn})
    return in_maps


def _run(x, weight, bias, trace=False):
    from concourse.bass_utils import run_bass_kernel_spmd

    nc = _build()
    in_maps = _prep_inputs(x, weight)
    res = run_bass_kernel_spmd(nc, in_maps, core_ids=list(range(NCORES)),
                               trace=trace)
    out = np.concatenate([res.results[c]["out"] for c in range(NCORES)],
                         axis=0)
    bias = np.asarray(bias, dtype=np.float32)
    if np.any(bias):
        out += bias
    return out.reshape(B, S, D_OUT), res


def kernel(x, weight, bias):
    out, _ = _run(x, weight, bias, trace=False)
    return out


# revision 11
# speedup vs baseline: 1.0213x; 1.0003x over previous
"""BitNet-style row-parallel linear on 8 TRN2 NeuronCores.

Reference computes: out[b,s,o] = sum_d x[b,s,d] * sign(w[o,d]) + bias[o]
  x: [4, 2048, 4096] f32, w: [4096, 4096] f32, bias: [4096] f32.

Strategy: data-parallel over the 8192 (b*s) rows — each of the 8 cores
computes a 1024-row slice of the output against the full binarized
weight. No collective needed; shards concatenate to the full output.
(The row-parallel/all-reduce hint costs a 128MB all-reduce per core;
sharding M instead makes the partial outputs disjoint.)

TensorE consumes both operands K-major, so the host preps:
  kxm = x_shard.T           [K=4096, M=1024]  (per core)
  kxn = sign(w).T           [K=4096, N=4096]  (same on every core)
Matmul runs in float32r (fp22 multiply, fp32 accumulate) — 4x faster
than true fp32 on the PE and far more accurate than bf16 inputs.
"""

import numpy as np

B, S, D_IN, D_OUT = 4, 2048, 4096, 4096
NCORES = 8
M_TOTAL = B * S
M_CORE = M_TOTAL // NCORES

import os

_cache = {}

# "f32r" (fp22 multiply, highest precision) or "bf16" (half the DMA
# traffic + fast weight load; weights are exactly representable).
DTYPE = os.environ.get("BK_DTYPE", "bf16")


IMPL = os.environ.get("BK_IMPL", "custom")


def _custom_body(nc, tc, kxm, kxn, out, mm_dt, mybir):
    """x^T stays SBUF-resident; sign(w)^T streams through once.

    Per n-block of 512 output columns: accumulate all 32 k-tiles into
    8 PSUM banks (one per 128-row m-tile), in two halves of 4 banks so
    each half's eviction overlaps the other half's matmuls.
    """
    P = 128
    KT = D_IN // P          # 32 k tiles
    MT = M_CORE // P        # 8 m tiles
    NW = 512
    NB = D_OUT // NW        # 8 n blocks
    CH = 4                  # k tiles per kxn DMA chunk
    NC = KT // CH           # chunks per n block
    f32 = mybir.dt.float32

    from contextlib import ExitStack
    with ExitStack() as ctx:
        kxm_pool = ctx.enter_context(tc.tile_pool(name="kxm", bufs=1))
        kxn_pool = ctx.enter_context(tc.tile_pool(name="kxn", bufs=16))
        psum_pool = ctx.enter_context(
            tc.tile_pool(name="psum", bufs=8, space="PSUM"))
        out_pool = ctx.enter_context(tc.tile_pool(name="outp", bufs=8))

        def issue_chunks(nb):
            ncols = slice(nb * NW, (nb + 1) * NW)
            chunks = []
            for c in range(NC):
                t = kxn_pool.tile([P, CH, NW], mm_dt, tag="kxn",
                                  name=f"kxn_{nb}_{c}")
                src = kxn[c * CH * P:(c + 1) * CH * P, ncols]
                nc.sync.dma_start(
                    out=t, in_=src.rearrange("(ko ki) n -> ki ko n", ki=P))
                chunks.append(t)
            return chunks

        # Interleave the x loads (needed at k-loop pace) with block 0's
        # weight chunks, alternating the scalar/vector queues so no
        # single DMA queue head-of-line-blocks the stream.
        kxm_tiles = []

        def issue_kxm(k):
            kt = kxm_pool.tile([P, M_CORE], mm_dt, tag="kxm",
                               name=f"kxm_{k}", bufs=KT)
            eng = nc.scalar if k % 2 == 0 else nc.gpsimd
            eng.dma_start(out=kt[:, :], in_=kxm[k * P:(k + 1) * P, :])
            kxm_tiles.append(kt)

        issue_kxm(0)
        issue_kxm(1)
        ncols0 = slice(0, NW)
        next_chunks = []
        for c in range(NC):
            t = kxn_pool.tile([P, CH, NW], mm_dt, tag="kxn",
                              name=f"kxn_0_{c}")
            src = kxn[c * CH * P:(c + 1) * CH * P, ncols0]
            nc.sync.dma_start(
                out=t, in_=src.rearrange("(ko ki) n -> ki ko n", ki=P))
            next_chunks.append(t)
            for k in range(2 + c * 4, min(2 + (c + 1) * 4, KT)):
                issue_kxm(k)
        for k in range(2 + NC * 4, KT):
            issue_kxm(k)

        for nb in range(NB):
            ncols = slice(nb * NW, (nb + 1) * NW)
            chunks = next_chunks
            psums = [psum_pool.tile([P, NW], f32, tag="ps", name=f"ps_{nb}_{i}")
                     for i in range(MT)]
            for half in range(2):
                ms = range(half * (MT // 2), (half + 1) * (MT // 2))
                for k in range(KT):
                    rhs = chunks[k // CH][:, k % CH, :]
                    for m in ms:
                        nc.tensor.matmul(
                            psums[m][:, :],
                            lhsT=kxm_tiles[k][:, m * P:(m + 1) * P],
                            rhs=rhs,
                            start=(k == 0), stop=(k == KT - 1))
                if half == 0 and nb + 1 < NB:
                    next_chunks = issue_chunks(nb + 1)
                for m in ms:
                    ot = out_pool.tile([P, NW], f32, tag="ot", name=f"ot_{nb}_{m}")
                    nc.vector.tensor_copy(out=ot[:, :], in_=psums[m][:, :])
                    nc.gpsimd.dma_start(
                        out=out[m * P:(m + 1) * P, ncols], in_=ot[:, :])


def _build():
    """Build + compile the 8-core SPMD Bass program once per process."""
    if "nc" in _cache:
        return _cache["nc"]

    import concourse.bacc as bacc
    import concourse.tile as tile
    import concourse.mybir as mybir
    from concourse.kernels.tile_matmul import matmul_tile_kernel

    mm_dt = {"f32r": mybir.dt.float32r, "bf16": mybir.dt.bfloat16}[DTYPE]

    nc = bacc.Bacc("TRN2", target_bir_lowering=False, debug=False,
                   num_devices=NCORES)
    kxm = nc.dram_tensor("kxm", [D_IN, M_CORE], mm_dt,
                         kind="ExternalInput").ap()
    kxn = nc.dram_tensor("kxn", [D_IN, D_OUT], mm_dt,
                         kind="ExternalInput").ap()
    out = nc.dram_tensor("out", [M_CORE, D_OUT], mybir.dt.float32,
                         kind="ExternalOutput").ap()
    if IMPL == "custom":
        with tile.TileContext(nc) as tc:
            _custom_body(nc, tc, kxm, kxn, out, mm_dt, mybir)
    else:
        kw = {}
        if os.environ.get("BK_MAX_K_TILE"):
            kw["MAX_K_TILE_SIZE"] = int(os.environ["BK_MAX_K_TILE"])
        if os.environ.get("BK_SKIP_K_SNAKE"):
            kw["skip_k_snake"] = True
        if os.environ.get("BK_NO_CACHE_TILES"):
            kw["cache_tiles"] = False
        with tile.TileContext(nc) as tc:
            matmul_tile_kernel(tc, kxm, kxn, out, **kw)
    nc.compile()
    _cache["nc"] = nc
    return nc


def _prep_inputs(x, weight):
    if DTYPE == "bf16":
        import ml_dtypes
        np_dt = ml_dtypes.bfloat16
    else:
        np_dt = np.float32
    x2d = np.asarray(x, dtype=np.float32).reshape(M_TOTAL, D_IN)
    kxn = np.ascontiguousarray(np.sign(weight, dtype=np.float32).T.astype(np_dt))
    in_maps = []
    for c in range(NCORES):
        kxm = np.ascontiguousarray(x2d[c * M_CORE:(c + 1) * M_CORE].T.astype(np_dt))
        in_maps.append({"kxm": kxm, "kxn": kxn})
    return in_maps


def _run(x, weight, bias, trace=False):
    from concourse.bass_utils import run_bass_kernel_spmd

    nc = _build()
    in_maps = _prep_inputs(x, weight)
    res = run_bass_kernel_spmd(nc, in_maps, core_ids=list(range(NCORES)),
                               trace=trace)
    out = np.concatenate([res.results[c]["out"] for c in range(NCORES)],
                         axis=0)
    bias = np.asarray(bias, dtype=np.float32)
    if np.any(bias):
        out += bias
    return out.reshape(B, S, D_OUT), res


def kernel(x, weight, bias):
    out, _ = _run(x, weight, bias, trace=False)
    return out


# revision 12
# speedup vs baseline: 1.0396x; 1.0180x over previous
"""BitNet-style row-parallel linear on 8 TRN2 NeuronCores.

Reference computes: out[b,s,o] = sum_d x[b,s,d] * sign(w[o,d]) + bias[o]
  x: [4, 2048, 4096] f32, w: [4096, 4096] f32, bias: [4096] f32.

Strategy: data-parallel over the 8192 (b*s) rows — each of the 8 cores
computes a 1024-row slice of the output against the full binarized
weight. No collective needed; shards concatenate to the full output.
(The row-parallel/all-reduce hint costs a 128MB all-reduce per core;
sharding M instead makes the partial outputs disjoint.)

TensorE consumes both operands K-major, so the host preps:
  kxm = x_shard.T           [K=4096, M=1024]  (per core)
  kxn = sign(w).T           [K=4096, N=4096]  (same on every core)
Matmul runs in float32r (fp22 multiply, fp32 accumulate) — 4x faster
than true fp32 on the PE and far more accurate than bf16 inputs.
"""

import numpy as np

B, S, D_IN, D_OUT = 4, 2048, 4096, 4096
NCORES = 8
M_TOTAL = B * S
M_CORE = M_TOTAL // NCORES

import os

_cache = {}

# "f32r" (fp22 multiply, highest precision) or "bf16" (half the DMA
# traffic + fast weight load; weights are exactly representable).
DTYPE = os.environ.get("BK_DTYPE", "bf16")


IMPL = os.environ.get("BK_IMPL", "custom")


def _custom_body(nc, tc, kxm, kxn, out, mm_dt, mybir):
    """x^T stays SBUF-resident; sign(w)^T streams through once.

    Per n-block of 512 output columns: accumulate all 32 k-tiles into
    8 PSUM banks (one per 128-row m-tile), in two halves of 4 banks so
    each half's eviction overlaps the other half's matmuls.
    """
    P = 128
    KT = D_IN // P          # 32 k tiles
    MT = M_CORE // P        # 8 m tiles
    NW = 512
    NB = D_OUT // NW        # 8 n blocks
    CH = 4                  # k tiles per kxn DMA chunk
    NC = KT // CH           # chunks per n block
    f32 = mybir.dt.float32

    from contextlib import ExitStack
    with ExitStack() as ctx:
        kxm_pool = ctx.enter_context(tc.tile_pool(name="kxm", bufs=1))
        kxn_pool = ctx.enter_context(tc.tile_pool(name="kxn", bufs=16))
        psum_pool = ctx.enter_context(
            tc.tile_pool(name="psum", bufs=8, space="PSUM"))
        out_pool = ctx.enter_context(tc.tile_pool(name="outp", bufs=8))

        def issue_chunks(nb):
            ncols = slice(nb * NW, (nb + 1) * NW)
            chunks = []
            for c in range(NC):
                t = kxn_pool.tile([P, CH, NW], mm_dt, tag="kxn",
                                  name=f"kxn_{nb}_{c}")
                src = kxn[c * CH * P:(c + 1) * CH * P, ncols]
                nc.sync.dma_start(
                    out=t, in_=src.rearrange("(ko ki) n -> ki ko n", ki=P))
                chunks.append(t)
            return chunks

        # Interleave the x loads (needed at k-loop pace) with block 0's
        # weight chunks, alternating the scalar/vector queues so no
        # single DMA queue head-of-line-blocks the stream.
        kxm_tiles = []

        def issue_kxm(k):
            kt = kxm_pool.tile([P, M_CORE], mm_dt, tag="kxm",
                               name=f"kxm_{k}", bufs=KT)
            eng = nc.scalar if k % 2 == 0 else nc.gpsimd
            eng.dma_start(out=kt[:, :], in_=kxm[k * P:(k + 1) * P, :])
            kxm_tiles.append(kt)

        issue_kxm(0)
        issue_kxm(1)
        ncols0 = slice(0, NW)
        next_chunks = []
        for c in range(NC):
            t = kxn_pool.tile([P, CH, NW], mm_dt, tag="kxn",
                              name=f"kxn_0_{c}")
            src = kxn[c * CH * P:(c + 1) * CH * P, ncols0]
            nc.sync.dma_start(
                out=t, in_=src.rearrange("(ko ki) n -> ki ko n", ki=P))
            next_chunks.append(t)
            for k in range(2 + c * 4, min(2 + (c + 1) * 4, KT)):
                issue_kxm(k)
        for k in range(2 + NC * 4, KT):
            issue_kxm(k)

        for nb in range(NB):
            ncols = slice(nb * NW, (nb + 1) * NW)
            chunks = next_chunks
            psums = [psum_pool.tile([P, NW], f32, tag="ps", name=f"ps_{nb}_{i}")
                     for i in range(MT)]
            # Block 0 runs while x is still streaming in: sweep all 8
            # banks per k-tile so each x tile is needed only every
            # ~1.7us. Later blocks have x resident, so split into two
            # 4-bank halves and overlap each half's eviction with the
            # other half's matmuls.
            groups = [range(MT)] if nb == 0 else [
                range(MT // 2), range(MT // 2, MT)]
            for gi, ms in enumerate(groups):
                for k in range(KT):
                    rhs = chunks[k // CH][:, k % CH, :]
                    for m in ms:
                        nc.tensor.matmul(
                            psums[m][:, :],
                            lhsT=kxm_tiles[k][:, m * P:(m + 1) * P],
                            rhs=rhs,
                            start=(k == 0), stop=(k == KT - 1))
                if gi == 0 and nb + 1 < NB:
                    next_chunks = issue_chunks(nb + 1)
                for m in ms:
                    ot = out_pool.tile([P, NW], f32, tag="ot", name=f"ot_{nb}_{m}")
                    nc.vector.tensor_copy(out=ot[:, :], in_=psums[m][:, :])
                    nc.gpsimd.dma_start(
                        out=out[m * P:(m + 1) * P, ncols], in_=ot[:, :])


def _build():
    """Build + compile the 8-core SPMD Bass program once per process."""
    if "nc" in _cache:
        return _cache["nc"]

    import concourse.bacc as bacc
    import concourse.tile as tile
    import concourse.mybir as mybir
    from concourse.kernels.tile_matmul import matmul_tile_kernel

    mm_dt = {"f32r": mybir.dt.float32r, "bf16": mybir.dt.bfloat16}[DTYPE]

    nc = bacc.Bacc("TRN2", target_bir_lowering=False, debug=False,
                   num_devices=NCORES)
    kxm = nc.dram_tensor("kxm", [D_IN, M_CORE], mm_dt,
                         kind="ExternalInput").ap()
    kxn = nc.dram_tensor("kxn", [D_IN, D_OUT], mm_dt,
                         kind="ExternalInput").ap()
    out = nc.dram_tensor("out", [M_CORE, D_OUT], mybir.dt.float32,
                         kind="ExternalOutput").ap()
    if IMPL == "custom":
        with tile.TileContext(nc) as tc:
            _custom_body(nc, tc, kxm, kxn, out, mm_dt, mybir)
    else:
        kw = {}
        if os.environ.get("BK_MAX_K_TILE"):
            kw["MAX_K_TILE_SIZE"] = int(os.environ["BK_MAX_K_TILE"])
        if os.environ.get("BK_SKIP_K_SNAKE"):
            kw["skip_k_snake"] = True
        if os.environ.get("BK_NO_CACHE_TILES"):
            kw["cache_tiles"] = False
        with tile.TileContext(nc) as tc:
            matmul_tile_kernel(tc, kxm, kxn, out, **kw)
    nc.compile()
    _cache["nc"] = nc
    return nc


def _prep_inputs(x, weight):
    if DTYPE == "bf16":
        import ml_dtypes
        np_dt = ml_dtypes.bfloat16
    else:
        np_dt = np.float32
    x2d = np.asarray(x, dtype=np.float32).reshape(M_TOTAL, D_IN)
    kxn = np.ascontiguousarray(np.sign(weight, dtype=np.float32).T.astype(np_dt))
    in_maps = []
    for c in range(NCORES):
        kxm = np.ascontiguousarray(x2d[c * M_CORE:(c + 1) * M_CORE].T.astype(np_dt))
        in_maps.append({"kxm": kxm, "kxn": kxn})
    return in_maps


def _run(x, weight, bias, trace=False):
    from concourse.bass_utils import run_bass_kernel_spmd

    nc = _build()
    in_maps = _prep_inputs(x, weight)
    res = run_bass_kernel_spmd(nc, in_maps, core_ids=list(range(NCORES)),
                               trace=trace)
    out = np.concatenate([res.results[c]["out"] for c in range(NCORES)],
                         axis=0)
    bias = np.asarray(bias, dtype=np.float32)
    if np.any(bias):
        out += bias
    return out.reshape(B, S, D_OUT), res


def kernel(x, weight, bias):
    out, _ = _run(x, weight, bias, trace=False)
    return out


# revision 13
# speedup vs baseline: 1.0422x; 1.0024x over previous
"""BitNet-style row-parallel linear on 8 TRN2 NeuronCores.

Reference computes: out[b,s,o] = sum_d x[b,s,d] * sign(w[o,d]) + bias[o]
  x: [4, 2048, 4096] f32, w: [4096, 4096] f32, bias: [4096] f32.

Strategy: data-parallel over the 8192 (b*s) rows — each of the 8 cores
computes a 1024-row slice of the output against the full binarized
weight. No collective needed; shards concatenate to the full output.
(The row-parallel/all-reduce hint costs a 128MB all-reduce per core;
sharding M instead makes the partial outputs disjoint.)

TensorE consumes both operands K-major, so the host preps:
  kxm = x_shard.T           [K=4096, M=1024]  (per core)
  kxn = sign(w).T           [K=4096, N=4096]  (same on every core)
Matmul runs in float32r (fp22 multiply, fp32 accumulate) — 4x faster
than true fp32 on the PE and far more accurate than bf16 inputs.
"""

import numpy as np

B, S, D_IN, D_OUT = 4, 2048, 4096, 4096
NCORES = 8
M_TOTAL = B * S
M_CORE = M_TOTAL // NCORES

import os

_cache = {}

# "f32r" (fp22 multiply, highest precision) or "bf16" (half the DMA
# traffic + fast weight load; weights are exactly representable).
DTYPE = os.environ.get("BK_DTYPE", "bf16")


IMPL = os.environ.get("BK_IMPL", "custom")


def _custom_body(nc, tc, kxm, kxn, out, mm_dt, mybir):
    """x^T stays SBUF-resident; sign(w)^T streams through once.

    Per n-block of 512 output columns: accumulate all 32 k-tiles into
    8 PSUM banks (one per 128-row m-tile), in two halves of 4 banks so
    each half's eviction overlaps the other half's matmuls.
    """
    P = 128
    KT = D_IN // P          # 32 k tiles
    MT = M_CORE // P        # 8 m tiles
    NW = 512
    NB = D_OUT // NW        # 8 n blocks
    CH = 4                  # k tiles per kxn DMA chunk
    NC = KT // CH           # chunks per n block
    f32 = mybir.dt.float32

    from contextlib import ExitStack
    with ExitStack() as ctx:
        kxm_pool = ctx.enter_context(tc.tile_pool(name="kxm", bufs=1))
        kxn_pool = ctx.enter_context(tc.tile_pool(name="kxn", bufs=16))
        psum_pool = ctx.enter_context(
            tc.tile_pool(name="psum", bufs=8, space="PSUM"))
        out_pool = ctx.enter_context(tc.tile_pool(name="outp", bufs=8))

        def issue_chunks(nb):
            ncols = slice(nb * NW, (nb + 1) * NW)
            chunks = []
            for c in range(NC):
                t = kxn_pool.tile([P, CH, NW], mm_dt, tag="kxn",
                                  name=f"kxn_{nb}_{c}")
                src = kxn[c * CH * P:(c + 1) * CH * P, ncols]
                nc.sync.dma_start(
                    out=t, in_=src.rearrange("(ko ki) n -> ki ko n", ki=P))
                chunks.append(t)
            return chunks

        # Interleave the x loads (needed at k-loop pace) with block 0's
        # weight chunks, alternating the scalar/vector queues so no
        # single DMA queue head-of-line-blocks the stream.
        kxm_tiles = []

        def issue_kxm(k):
            kt = kxm_pool.tile([P, M_CORE], mm_dt, tag="kxm",
                               name=f"kxm_{k}", bufs=KT)
            eng = nc.scalar if k % 2 == 0 else nc.gpsimd
            eng.dma_start(out=kt[:, :], in_=kxm[k * P:(k + 1) * P, :])
            kxm_tiles.append(kt)

        issue_kxm(0)
        issue_kxm(1)
        ncols0 = slice(0, NW)
        next_chunks = []
        for c in range(NC):
            t = kxn_pool.tile([P, CH, NW], mm_dt, tag="kxn",
                              name=f"kxn_0_{c}")
            src = kxn[c * CH * P:(c + 1) * CH * P, ncols0]
            nc.sync.dma_start(
                out=t, in_=src.rearrange("(ko ki) n -> ki ko n", ki=P))
            next_chunks.append(t)
            for k in range(2 + c * 4, min(2 + (c + 1) * 4, KT)):
                issue_kxm(k)
        for k in range(2 + NC * 4, KT):
            issue_kxm(k)

        for nb in range(NB):
            ncols = slice(nb * NW, (nb + 1) * NW)
            chunks = next_chunks
            psums = [psum_pool.tile([P, NW], f32, tag="ps", name=f"ps_{nb}_{i}")
                     for i in range(MT)]
            # Block 0 runs while x is still streaming in: sweep all 8
            # banks per k-tile so each x tile is needed only every
            # ~1.7us. Later blocks have x resident, so run one bank at
            # a time (full k-sweep per m-tile): evictions trickle out
            # during compute and the post-loop tail is a single
            # evict+store.
            groups = [range(MT)] if nb == 0 else [[m] for m in range(MT)]
            for gi, ms in enumerate(groups):
                for k in range(KT):
                    rhs = chunks[k // CH][:, k % CH, :]
                    for m in ms:
                        nc.tensor.matmul(
                            psums[m][:, :],
                            lhsT=kxm_tiles[k][:, m * P:(m + 1) * P],
                            rhs=rhs,
                            start=(k == 0), stop=(k == KT - 1))
                if gi == 0 and nb + 1 < NB:
                    next_chunks = issue_chunks(nb + 1)
                for m in ms:
                    ot = out_pool.tile([P, NW], f32, tag="ot", name=f"ot_{nb}_{m}")
                    nc.vector.tensor_copy(out=ot[:, :], in_=psums[m][:, :])
                    nc.gpsimd.dma_start(
                        out=out[m * P:(m + 1) * P, ncols], in_=ot[:, :])


def _build():
    """Build + compile the 8-core SPMD Bass program once per process."""
    if "nc" in _cache:
        return _cache["nc"]

    import concourse.bacc as bacc
    import concourse.tile as tile
    import concourse.mybir as mybir
    from concourse.kernels.tile_matmul import matmul_tile_kernel

    mm_dt = {"f32r": mybir.dt.float32r, "bf16": mybir.dt.bfloat16}[DTYPE]

    nc = bacc.Bacc("TRN2", target_bir_lowering=False, debug=False,
                   num_devices=NCORES)
    kxm = nc.dram_tensor("kxm", [D_IN, M_CORE], mm_dt,
                         kind="ExternalInput").ap()
    kxn = nc.dram_tensor("kxn", [D_IN, D_OUT], mm_dt,
                         kind="ExternalInput").ap()
    out = nc.dram_tensor("out", [M_CORE, D_OUT], mybir.dt.float32,
                         kind="ExternalOutput").ap()
    if IMPL == "custom":
        with tile.TileContext(nc) as tc:
            _custom_body(nc, tc, kxm, kxn, out, mm_dt, mybir)
    else:
        kw = {}
        if os.environ.get("BK_MAX_K_TILE"):
            kw["MAX_K_TILE_SIZE"] = int(os.environ["BK_MAX_K_TILE"])
        if os.environ.get("BK_SKIP_K_SNAKE"):
            kw["skip_k_snake"] = True
        if os.environ.get("BK_NO_CACHE_TILES"):
            kw["cache_tiles"] = False
        with tile.TileContext(nc) as tc:
            matmul_tile_kernel(tc, kxm, kxn, out, **kw)
    nc.compile()
    _cache["nc"] = nc
    return nc


def _prep_inputs(x, weight):
    if DTYPE == "bf16":
        import ml_dtypes
        np_dt = ml_dtypes.bfloat16
    else:
        np_dt = np.float32
    x2d = np.asarray(x, dtype=np.float32).reshape(M_TOTAL, D_IN)
    kxn = np.ascontiguousarray(np.sign(weight, dtype=np.float32).T.astype(np_dt))
    in_maps = []
    for c in range(NCORES):
        kxm = np.ascontiguousarray(x2d[c * M_CORE:(c + 1) * M_CORE].T.astype(np_dt))
        in_maps.append({"kxm": kxm, "kxn": kxn})
    return in_maps


def _run(x, weight, bias, trace=False):
    from concourse.bass_utils import run_bass_kernel_spmd

    nc = _build()
    in_maps = _prep_inputs(x, weight)
    res = run_bass_kernel_spmd(nc, in_maps, core_ids=list(range(NCORES)),
                               trace=trace)
    out = np.concatenate([res.results[c]["out"] for c in range(NCORES)],
                         axis=0)
    bias = np.asarray(bias, dtype=np.float32)
    if np.any(bias):
        out += bias
    return out.reshape(B, S, D_OUT), res


def kernel(x, weight, bias):
    out, _ = _run(x, weight, bias, trace=False)
    return out
